# revision 84
# baseline (speedup 1.0000x reference)
"""BPLoss Trainium2 kernel: 8-core SPMD over the detection (N) axis.

v11 design: fp16 streaming + maskless block-max + host-gathered label value.

Per core (shard of R=12544 rows; partition p owns rows p*98..p*98+97):
  - class_scores uploaded as fp16 (half the HBM traffic), shifted and scaled
    on host to (cs - 1) * 1024 so the row max (~1 - 1e-3 for uniform scores)
    sits near magnitude ~1 with full fp16 relative precision instead of near
    1.0 where fp16 spacing (4.9e-4) would swamp log(max).  The device undoes
    it inside the Ln activation: log(masked/1024 + 1).
  - NO mask pass over the [N, 1024] matrix: a pairwise tensor_tensor-max
    tree (fp16 2x mode, 0.56 ns/elem vs reduce_max's 0.93) folds each
    1024-col row to 8 "class" maxes S[k] = max over columns == k (mod 8);
    3 levels per 7-tile DMA group, then 4 more levels in bulk tail chunks
    interleaved into the stream.
  - Masked row-max reconstruction (exact unless the 2nd-largest element
    shares its mod-8 class with the label AND the label is the argmax,
    ~25 rows in 100k, ~1e-3 log error each):
        M1 = max_k S[k]
        Vd = max_k (S[k] + P8[k])     with P8 = -4096 at the label's class
        masked = (v == M1) ? Vd : M1  computed as max(Vd, M1 - 1e9*(v==M1))
    where v = cs_fp16[row, label] is gathered on host (O(N), same class of
    host work as the gt-table lookups) and P8 is a host-built penalty table.
  - epilogue in 3 pieces (keyed to tail-chunk completion) so the serial
    chain after the last DMA group is short: Ln on ScalarE (table preloaded
    at kernel start), fused multiply-accumulate dots for
    sum((z+r)*log_masked) and sum(z*||xywh - gt_xywh[idx]||^2).
  - Scheduling regime (from v5-v10 traces): DVE consumes a 7-tile group in
    ~3.7us + ~0.8us of interleaved tail/epi work vs the DMA's ~4.5us
    delivery period.  The pf-blocked shape chain at the head delays DVE's
    first group by ~2 DMA periods, building a buffer backlog that keeps DVE
    busy gaplessly (a DVE data-wait costs ~1.5us wakeup latency per group,
    so pacing DVE *behind* the stream beats starting it early).
Host: gathers the tiny gt tables per row, shards, pads core 7, sums the
8x[128,4] partials, combines -A + exp(-B).
"""
import numpy as np
import concourse.bass as bass
import concourse.tile as tile
from concourse import bacc, mybir
from concourse.bass_utils import run_bass_kernel_spmd

N, C, M = 100000, 1024, 128
NCORES = 8
T = 98              # 128-row tiles per core
R = T * 128         # 12544 rows per core
G = 7               # tiles per DMA group
GROUPS = [(i * G, G) for i in range(14)]
NB = 8              # per-row classes (columns mod 8 after the fold tree)
CS_BUFS = 6
SCALE = 1024.0      # host uploads (cs - 1) * SCALE
PEN = -4096.0       # class penalty; dominates any real shifted score (>= -1024)

f16 = mybir.dt.float16
f32 = mybir.dt.float32
OP = mybir.AluOpType
AF = mybir.ActivationFunctionType
AX = mybir.AxisListType

# packed f32 per-row tables: [v | z | r | xywh | g]
PF_V = 0
PF_Z = T
PF_R = 2 * T
PF_XYWH = 3 * T
PF_G = 7 * T
PF_COLS = 11 * T


def build_nc(reps=1):
    nc = bacc.Bacc("TRN2", target_bir_lowering=False, debug=False,
                   num_devices=NCORES)
    cs_d = nc.dram_tensor("cs", [128, T * C], f16, kind="ExternalInput").ap()
    pf_d = nc.dram_tensor("pf", [128, PF_COLS], f32, kind="ExternalInput").ap()
    p8_d = nc.dram_tensor("p8", [128, T * NB], f16, kind="ExternalInput").ap()
    out_d = nc.dram_tensor("out", [128, 4], f32, kind="ExternalOutput").ap()

    with tile.TileContext(nc) as tc:
        with (
            tc.tile_pool(name="const", bufs=1) as constp,
            tc.tile_pool(name="csp", bufs=CS_BUFS) as csp,
            tc.tile_pool(name="vp", bufs=2) as vp,
        ):
            # pf/p8 on the scalar queue: the sync queue must stay a pure cs
            # stream (leading small DMAs there cost a ~2.7us engine gap).
            # pf lands ~2 cs-groups in; the DVE-scheduled-first shape chain
            # blocking on it is what builds the desired DVE backlog.
            pf = constp.tile([128, PF_COLS], f32)
            nc.scalar.dma_start(out=pf[:], in_=pf_d[:])
            p8 = constp.tile([128, T, NB], f16)
            nc.scalar.dma_start(out=p8[:], in_=p8_d[:])
            v_sb = pf[:, PF_V : PF_V + T]
            z_sb = pf[:, PF_Z : PF_Z + T]
            r_sb = pf[:, PF_R : PF_R + T]
            xywh_sb = pf[:, PF_XYWH : PF_XYWH + 4 * T].rearrange(
                "p (t c) -> p t c", c=4
            )
            g_sb = pf[:, PF_G : PF_G + 4 * T].rearrange("p (t c) -> p t c", c=4)

            S = constp.tile([128, T, NB], f16)
            S2 = constp.tile([128, T, NB], f16)
            B3 = constp.tile([128, T, 128], f16)
            w4t = constp.tile([128, T, 64], f16)
            w5t = constp.tile([128, T, 32], f16)
            w6t = constp.tile([128, T, 16], f16)
            m1 = constp.tile([128, T], f32)
            vd = constp.tile([128, T], f32)
            eq = constp.tile([128, T], f32)
            dd = constp.tile([128, T], f32)
            mm = constp.tile([128, T], f32)
            lm = constp.tile([128, T], f32)
            w_sb = constp.tile([128, T], f32)
            scr = constp.tile([128, T], f32)
            scr2 = constp.tile([128, T], f32)
            diff = constp.tile([128, T, 4], f32)
            dsum = constp.tile([128, T], f32)
            out_sb = constp.tile([128, 4], f32)
            warm = constp.tile([128, 1], f32)
            warm2 = constp.tile([128, 1], f32)

            # preload the Ln activation table while DMAs stream
            nc.vector.memset(warm[:], 1.0)
            nc.scalar.activation(out=warm2[:], in_=warm[:], func=AF.Ln)
            nc.vector.memset(out_sb[:], 0.0)

            for rep in range(reps):

                def shape_term():
                    """sum(z * ||xywh - gt||^2): independent of class_scores."""
                    nc.vector.tensor_add(w_sb[:], z_sb, r_sb)
                    nc.vector.tensor_sub(diff[:], xywh_sb, g_sb)
                    nc.vector.tensor_mul(diff[:], diff[:], diff[:])
                    nc.vector.reduce_sum(dsum[:], diff[:], axis=AX.X)
                    nc.vector.scalar_tensor_tensor(
                        out=scr2[:], in0=z_sb, scalar=0.0, in1=dsum[:],
                        op0=OP.bypass, op1=OP.mult, accum_out=out_sb[:, 3:4],
                    )

                def tail(ta, tb):
                    """tree levels 4-7 + per-row S for tiles [ta, tb)."""
                    nc.vector.tensor_tensor(
                        out=w4t[:, ta:tb, :], in0=B3[:, ta:tb, 0:64],
                        in1=B3[:, ta:tb, 64:128], op=OP.max,
                    )
                    nc.vector.tensor_tensor(
                        out=w5t[:, ta:tb, :], in0=w4t[:, ta:tb, 0:32],
                        in1=w4t[:, ta:tb, 32:64], op=OP.max,
                    )
                    nc.vector.tensor_tensor(
                        out=w6t[:, ta:tb, :], in0=w5t[:, ta:tb, 0:16],
                        in1=w5t[:, ta:tb, 16:32], op=OP.max,
                    )
                    nc.vector.tensor_tensor(
                        out=S[:, ta:tb, :], in0=w6t[:, ta:tb, 0:8],
                        in1=w6t[:, ta:tb, 8:16], op=OP.max,
                    )

                def epilogue(ta, tb, col):
                    """masked row-max + log dot for tiles [ta, tb)."""
                    nc.vector.reduce_max(
                        m1[:, ta:tb], S[:, ta:tb, :], axis=AX.X
                    )
                    nc.vector.tensor_add(
                        S2[:, ta:tb, :], S[:, ta:tb, :], p8[:, ta:tb, :]
                    )
                    nc.vector.reduce_max(
                        vd[:, ta:tb], S2[:, ta:tb, :], axis=AX.X
                    )
                    # masked = max(Vd, M1 - BIG*(v==M1)); Vd <= M1 always
                    nc.vector.tensor_tensor(
                        out=eq[:, ta:tb], in0=v_sb[:, ta:tb],
                        in1=m1[:, ta:tb], op=OP.is_equal,
                    )
                    nc.vector.scalar_tensor_tensor(
                        out=dd[:, ta:tb], in0=eq[:, ta:tb], scalar=-1.0e9,
                        in1=m1[:, ta:tb], op0=OP.mult, op1=OP.add,
                    )
                    nc.vector.tensor_tensor(
                        out=mm[:, ta:tb], in0=dd[:, ta:tb],
                        in1=vd[:, ta:tb], op=OP.max,
                    )
                    nc.scalar.activation(
                        out=lm[:, ta:tb], in_=mm[:, ta:tb], func=AF.Ln,
                        bias=1.0, scale=1.0 / SCALE,
                    )
                    nc.vector.scalar_tensor_tensor(
                        out=scr[:, ta:tb], in0=w_sb[:, ta:tb], scalar=0.0,
                        in1=lm[:, ta:tb], op0=OP.bypass, op1=OP.mult,
                        accum_out=out_sb[:, col : col + 1],
                    )

                # tail chunks / epilogue pieces keyed on group completion:
                # after group g, tiles [0, (g+1)*G) are in B3.
                tails = {3: (0, 28), 6: (28, 49), 10: (49, 77), 13: (77, 98)}
                epis = {7: (0, 49, 0)}

                # DMAs issue in stream order 0..13; DVE consumes group 1
                # before group 0 so it starts with ~2 groups of backlog
                # (a DVE data-wait costs ~1.5us wakeup latency, so backlog
                # beats promptness).
                csws = []
                for g, (t0, sz) in enumerate(GROUPS):
                    csw = csp.tile([128, G, C], f16)
                    nc.sync.dma_start(
                        out=csw[:, 0:sz, :],
                        in_=cs_d[:, t0 * C : (t0 + sz) * C],
                    )
                    csws.append(csw)
                    compute_order = (
                        [2, 1, 0] if g == 2 else [g] if g >= 3 else []
                    )
                    for gc in compute_order:
                        tc0, csz = GROUPS[gc]
                        cw = csws[gc]
                        w1 = vp.tile([128, G, 512], f16)
                        nc.vector.tensor_tensor(
                            out=w1[:, 0:csz, :], in0=cw[:, 0:csz, 0:512],
                            in1=cw[:, 0:csz, 512:1024], op=OP.max,
                        )
                        w2 = vp.tile([128, G, 256], f16)
                        nc.vector.tensor_tensor(
                            out=w2[:, 0:csz, :], in0=w1[:, 0:csz, 0:256],
                            in1=w1[:, 0:csz, 256:512], op=OP.max,
                        )
                        nc.vector.tensor_tensor(
                            out=B3[:, tc0 : tc0 + csz, :],
                            in0=w2[:, 0:csz, 0:128],
                            in1=w2[:, 0:csz, 128:256], op=OP.max,
                        )
                    if g == 4:
                        shape_term()
                    if g in tails:
                        tail(*tails[g])
                    if g in epis:
                        epilogue(*epis[g])
                epilogue(49, T, 1)
            nc.scalar.dma_start(out=out_d[:], in_=out_sb[:])

    nc.compile()
    return nc


def make_in_maps(class_scores, xywh, z, r, nearest_gt_idx, gt_class_labels, gt_xywh):
    cs_f = np.ascontiguousarray(np.asarray(class_scores, dtype=np.float32))
    cs16 = ((cs_f - 1.0) * SCALE).astype(np.float16)
    xywh = np.ascontiguousarray(np.asarray(xywh, dtype=np.float32))
    z = np.ascontiguousarray(np.asarray(z, dtype=np.float32))
    r = np.ascontiguousarray(np.asarray(r, dtype=np.float32))
    idx = np.asarray(nearest_gt_idx).astype(np.int64)
    labels = np.asarray(gt_class_labels).astype(np.int64)[idx]           # [N]
    gx = np.asarray(gt_xywh, dtype=np.float32)[idx]                      # [N,4]
    v = cs16[np.arange(N), labels].astype(np.float32)                    # [N]
    # class k of S = columns congruent to k mod 8 (pairwise fold halves the
    # column index range each tree level)
    kstar = (labels & 7).astype(np.int64)                                # [N]
    p8_full = np.zeros((N, NB), dtype=np.float16)
    p8_full[np.arange(N), kstar] = PEN

    in_maps = []
    for c in range(NCORES):
        lo, hi = c * R, (c + 1) * R
        if hi <= N:
            cs_s = cs16[lo:hi]
            v_s, z_s, r_s = v[lo:hi], z[lo:hi], r[lo:hi]
            xywh_s, gx_s, p8_s = xywh[lo:hi], gx[lo:hi], p8_full[lo:hi]
        else:
            n_real = N - lo
            cs_s = np.zeros((R, C), dtype=np.float16)   # pad rows: cs'=0 -> log1p(0)=0
            cs_s[:n_real] = cs16[lo:]
            v_s = np.zeros(R, np.float32); v_s[:n_real] = v[lo:]
            z_s = np.zeros(R, np.float32); z_s[:n_real] = z[lo:]
            r_s = np.zeros(R, np.float32); r_s[:n_real] = r[lo:]
            xywh_s = np.zeros((R, 4), np.float32); xywh_s[:n_real] = xywh[lo:]
            gx_s = np.zeros((R, 4), np.float32); gx_s[:n_real] = gx[lo:]
            p8_s = np.zeros((R, NB), np.float16)
            p8_s[:, 0] = PEN
            p8_s[:n_real] = p8_full[lo:]
        pf = np.empty((128, PF_COLS), dtype=np.float32)
        pf[:, PF_V : PF_V + T] = v_s.reshape(128, T)
        pf[:, PF_Z : PF_Z + T] = z_s.reshape(128, T)
        pf[:, PF_R : PF_R + T] = r_s.reshape(128, T)
        pf[:, PF_XYWH : PF_XYWH + 4 * T] = xywh_s.reshape(128, 4 * T)
        pf[:, PF_G : PF_G + 4 * T] = gx_s.reshape(128, 4 * T)
        in_maps.append({
            "cs": np.ascontiguousarray(cs_s.reshape(128, T * C)),
            "pf": pf,
            "p8": np.ascontiguousarray(p8_s.reshape(128, T * NB)),
        })
    return in_maps


def combine_outputs(outs):
    """outs: list of [128, 4] per-core partials -> final [1] float32."""
    partA = float(sum(o[:, 0:3].astype(np.float64).sum() for o in outs))
    partB = float(sum(o[:, 3].astype(np.float64).sum() for o in outs))
    with np.errstate(over="ignore", under="ignore"):
        tps = np.exp(-partB)
    val = -partA + tps
    return np.array([val], dtype=np.float32)


_NC_CACHE = None


def get_nc():
    global _NC_CACHE
    if _NC_CACHE is None:
        _NC_CACHE = build_nc()
    return _NC_CACHE


def kernel(**inputs) -> np.ndarray:
    nc = get_nc()
    in_maps = make_in_maps(**inputs)
    res = run_bass_kernel_spmd(nc, in_maps, core_ids=list(range(NCORES)))
    return combine_outputs([res.results[c]["out"] for c in range(NCORES)])
